# revision 1
# baseline (speedup 1.0000x reference)
"""DNC forward (single step) on 8 NeuronCores — Bass/Tile kernel.

Data parallel: 16 batches -> 2 per core. Key algebraic facts exploited
(valid for the prev_state==None path of the reference):

* prev_rw is uniform (1/N)  => fwd/bwd temporal read weights only need the
  row-sums and column-sums of L_new, never L_new itself.  With
  rowsum0 = L@1, Lw = L@w, colsum0 = 1@L, cw = w@L (w = write weights):
      rowsum_Lnew = (1-w)*rowsum0 - Lw + w*(sum(p) - p)
      colsum_Lnew = (1-w)*colsum0 - cw + p*(sum(w) - w)
  so L is streamed exactly once from HBM (the memory-bound roofline).
* var_phi / usage are constant across slots => argsort is the identity and
  allocation[n] = (1-u) * u^(n+1) with u = 1e-4 * prod_r(1 - free_gate_r/N).

Per 1 MB row-block of L (128 rows x 2048 cols) the four reductions run on
three different engines concurrently with the DMA stream:
  PE:  [1,w]^T @ block                       -> colsum0/cw (psum accumulate)
  DVE: tensor_tensor_reduce(block * w_bcast) -> Lw
  ACT: activation(Copy, accum_out)           -> rowsum0
All slot-indexed vectors live in a (128 partitions x 16 chunks) layout.
"""
import numpy as np
from contextlib import ExitStack

import concourse.bass as bass
import concourse.bacc as bacc
import concourse.tile as tile
from concourse import mybir
from concourse.bass_utils import run_bass_kernel_spmd

F32 = mybir.dt.float32
BF16 = mybir.dt.bfloat16
AF = mybir.ActivationFunctionType
OP = mybir.AluOpType

NCORES = 8
BC = 2                  # batches per core
N = 2048                # memory slots
NCH = N // 128          # 16 slot chunks
WD = 64                 # word size
R = 4                   # read heads
IN_D, H_D, IFACE = 256, 512, 727
EPS = 1e-8

# interface vector slice offsets
O_RK, O_RS, O_WK, O_WS = 0, 256, 260, 324
O_ER, O_WV, O_FG, O_AG, O_WG, O_RM = 325, 389, 453, 457, 458, 459


def _build_pre(nc, pools, aps, b):
    """Pre-L phase: controller, write addressing, memory update, read keys."""
    (bpool, bfat, lpool, scr_ttr, scr_act, scr64, pss, pbig, lbf, consts) = pools
    ones_row, ones_col, one_one, i128, iota, ones64, ones_row_bf = consts
    x_ap, mem_ap, l_ap, p_ap, out_ap = (
        aps['x'], aps['memory'], aps['L'], aps['p'], aps['out'])
    w1_sb, w2_sb, b1_sb, b2_sb = aps['w1_sb'], aps['w2_sb'], aps['b1_sb'], aps['b2_sb']

    act = nc.scalar
    dve = nc.vector
    gp = nc.gpsimd
    pe = nc.tensor

    def mm(out, lhsT, rhs, start=True, stop=True):
        pe.matmul(out, lhsT, rhs, start=start, stop=stop)

    def ps_small(p_, f):
        return pss.tile([p_, f], F32, tag="pss", name="pss")

    def sb(p_, f, tag):
        return bpool.tile([p_, f], F32, tag=tag, name=tag)

    def fat(p_, f, tag):
        return bfat.tile([p_, f], F32, tag=tag, name=tag)

    # -------- controller --------
    xb = sb(1, IN_D, "xb")
    nc.sync.dma_start(xb[:], x_ap[b:b + 1, :])

    xT = sb(128, 2, "xT")
    ptx = ps_small(128, 2)
    for c in range(2):
        mm(ptx[:, c:c + 1], xb[0:1, 128 * c:128 * (c + 1)], one_one[:])
    dve.tensor_copy(xT[:], ptx[:])

    h_ps = ps_small(1, H_D)
    for c in range(2):
        mm(h_ps[:], xT[:, c:c + 1], w1_sb[:, c, :], start=(c == 0), stop=(c == 1))
    h_lin = sb(1, H_D, "h_lin")
    dve.tensor_tensor(h_lin[:], h_ps[:], b1_sb[:], op=OP.add)
    h_sb = sb(1, H_D, "h_sb")
    act.activation(h_sb[:], h_lin[:], AF.Tanh)

    hT = sb(128, 4, "hT")
    pth = ps_small(128, 4)
    for c in range(4):
        mm(pth[:, c:c + 1], h_sb[0:1, 128 * c:128 * (c + 1)], one_one[:])
    dve.tensor_copy(hT[:], pth[:])

    v_sb = sb(1, IFACE, "v_sb")
    for lo, hi in ((0, 512), (512, IFACE)):
        v_ps = ps_small(1, hi - lo)
        for c in range(4):
            mm(v_ps[:], hT[:, c:c + 1], w2_sb[:, c, lo:hi],
               start=(c == 0), stop=(c == 3))
        dve.tensor_tensor(v_sb[0:1, lo:hi], v_ps[:], b2_sb[0:1, lo:hi], op=OP.add)

    # -------- interface nonlinearities --------
    er_sg = sb(1, WD, "er_sg")
    act.activation(er_sg[:], v_sb[0:1, O_ER:O_ER + WD], AF.Sigmoid)
    fg_sg = sb(1, R, "fg_sg")
    act.activation(fg_sg[:], v_sb[0:1, O_FG:O_FG + R], AF.Sigmoid)
    ag_sg = sb(1, 1, "ag_sg")
    act.activation(ag_sg[:], v_sb[0:1, O_AG:O_AG + 1], AF.Sigmoid)
    wg_sg = sb(1, 1, "wg_sg")
    act.activation(wg_sg[:], v_sb[0:1, O_WG:O_WG + 1], AF.Sigmoid)

    rs_s = sb(1, R, "rs_s")         # 1 + softplus(read strengths)
    act.activation(rs_s[:], v_sb[0:1, O_RS:O_RS + R], AF.Exp)
    act.activation(rs_s[:], rs_s[:], AF.Ln, bias=1.0)
    act.activation(rs_s[:], rs_s[:], AF.Copy, bias=1.0)
    ws_s = sb(1, 1, "ws_s")
    act.activation(ws_s[:], v_sb[0:1, O_WS:O_WS + 1], AF.Exp)
    act.activation(ws_s[:], ws_s[:], AF.Ln, bias=1.0)
    act.activation(ws_s[:], ws_s[:], AF.Copy, bias=1.0)

    rm_e = sb(1, 3 * R, "rm_e")
    act.activation(rm_e[:], v_sb[0:1, O_RM:O_RM + 3 * R], AF.Exp)
    rm_sum = sb(1, R, "rm_sum")
    dve.tensor_reduce(rm_sum[:], rm_e[:].rearrange("o (r t) -> o r t", t=3),
                      axis=mybir.AxisListType.X, op=OP.add)
    rm_rec = sb(1, R, "rm_rec")
    dve.reciprocal(rm_rec[:], rm_sum[:])
    modes = sb(1, 3 * R, "modes")
    dve.tensor_tensor(modes[:].rearrange("o (r t) -> o r t", t=3),
                      rm_e[:].rearrange("o (r t) -> o r t", t=3),
                      rm_rec[:].rearrange("o (r t) -> o r t", t=1)
                      .broadcast_to([1, R, 3]),
                      op=OP.mult)

    # -------- usage scalar u, allocation params --------
    fgN = sb(1, R, "fgN")
    act.activation(fgN[:], fg_sg[:], AF.Copy, scale=-1.0 / N, bias=1.0)
    fg2 = sb(1, 2, "fg2")
    dve.tensor_tensor(fg2[:], fgN[0:1, 0:2], fgN[0:1, 2:4], op=OP.mult)
    prod = sb(1, 1, "prod")
    dve.tensor_tensor(prod[:], fg2[0:1, 0:1], fg2[0:1, 1:2], op=OP.mult)
    u_sb = sb(1, 1, "u_sb")
    act.activation(u_sb[:], prod[:], AF.Copy, scale=1e-4)
    ln_u = sb(1, 1, "ln_u")
    act.activation(ln_u[:], u_sb[:], AF.Ln)
    omu = sb(1, 1, "omu")
    act.activation(omu[:], u_sb[:], AF.Copy, scale=-1.0, bias=1.0)

    # -------- memory load + row norms --------
    M_sb = bfat.tile([128, NCH * WD], F32, tag="M_sb", name="M_sb",
                      bufs=1)
    M3 = M_sb[:].rearrange("q (i w) -> q i w", w=WD)
    nc.sync.dma_start(M3, mem_ap[b].rearrange("(i q) w -> q i w", q=128))

    msq = sb(128, NCH, "msq")
    sq1 = scr_act.tile([128, NCH * WD], F32, tag="sact", name="sact")
    dve.tensor_tensor(sq1[:], M_sb[:], M_sb[:], op=OP.mult)
    dve.tensor_reduce(msq[:], sq1[:].rearrange(
        "q (i w) -> q i w", w=WD), axis=mybir.AxisListType.X, op=OP.add)
    mn_s = sb(128, NCH, "mn_s")
    act.activation(mn_s[:], msq[:], AF.Sqrt)
    dve.tensor_scalar_add(mn_s[:], mn_s[:], EPS)
    rn_w = sb(128, NCH, "rn_w")
    dve.reciprocal(rn_w[:], mn_s[:])

    # -------- write key normalization + content scores (gpsimd dot) --------
    wk2 = sb(1, 1, "wk2")
    s64b = scr64.tile([128, WD], F32, tag="s64", name="s64")
    act.activation(s64b[0:1, :], v_sb[0:1, O_WK:O_WK + WD], AF.Square,
                   accum_out=wk2[:])
    nk = sb(1, 1, "nk")
    act.activation(nk[:], wk2[:], AF.Sqrt)
    snk = sb(1, 1, "snk")
    dve.tensor_tensor(snk[:], ws_s[:], nk[:], op=OP.mult)
    act.activation(snk[:], snk[:], AF.Copy, bias=EPS)
    srec = sb(1, 1, "srec")
    dve.reciprocal(srec[:], snk[:])
    wf = sb(1, 1, "wf")
    dve.tensor_tensor(wf[:], ws_s[:], srec[:], op=OP.mult)
    kn = sb(1, WD, "kn")
    act.activation(kn[:], v_sb[0:1, O_WK:O_WK + WD], AF.Copy, scale=wf[:])
    kn_bc = sb(128, WD, "kn_bc")
    pt = ps_small(128, WD)
    mm(pt[:], ones_row[:], kn[:])
    dve.tensor_copy(kn_bc[:], pt[:])

    wsc_r = sb(128, NCH, "wsc_r")   # raw dot(M_n, kn) per slot
    for i in range(NCH):
        g64 = scr64.tile([128, WD], F32, tag="g64", name="g64")
        dve.scalar_tensor_tensor(out=g64[:], in0=M3[:, i, :], scalar=1.0,
                                 in1=kn_bc[:], op0=OP.mult, op1=OP.mult,
                                 accum_out=wsc_r[:, i:i + 1])
    wsc = sb(128, NCH, "wsc")
    dve.tensor_tensor(wsc[:], wsc_r[:], rn_w[:], op=OP.mult)

    # softmax over all 2048 slots
    wse = sb(128, NCH, "wse")
    wse_s = sb(128, 1, "wse_s")
    act.activation(wse[:], wsc[:], AF.Exp, accum_out=wse_s[:])
    ptt = ps_small(1, 1)
    mm(ptt[:], wse_s[:], ones_col[:])
    totr = sb(1, 1, "totr")
    dve.reciprocal(totr[:], ptt[:])

    # batch the per-batch scalars into one broadcast matmul:
    # [ln_u, 1-u, c1=wg*ag, c2=wg*(1-ag), 1/sum(exp(wsc))]
    omag = sb(1, 1, "omag")
    act.activation(omag[:], ag_sg[:], AF.Copy, scale=-1.0, bias=1.0)
    c1 = sb(1, 1, "c1")
    dve.tensor_tensor(c1[:], wg_sg[:], ag_sg[:], op=OP.mult)
    c2 = sb(1, 1, "c2")
    dve.tensor_tensor(c2[:], wg_sg[:], omag[:], op=OP.mult)
    sc5 = sb(1, 5, "sc5")
    for j, t in enumerate((ln_u, omu, c1, c2, totr)):
        dve.tensor_copy(sc5[0:1, j:j + 1], t[:])
    pb5 = ps_small(128, 5)
    mm(pb5[:], ones_row[:], sc5[:])
    scb = sb(128, 5, "scb")
    dve.tensor_copy(scb[:], pb5[:])

    # allocation = (1-u) * u^(n+1) and write weights
    alle = sb(128, NCH, "alle")
    act.activation(alle[:], iota[:], AF.Exp, scale=scb[:, 0:1])
    alloc = sb(128, NCH, "alloc")
    act.activation(alloc[:], alle[:], AF.Copy, scale=scb[:, 1:2])

    cww = sb(128, NCH, "cww")
    dve.tensor_scalar_mul(cww[:], wse[:], scb[:, 4:5])
    t2 = sb(128, NCH, "t2w")
    dve.tensor_scalar_mul(t2[:], cww[:], scb[:, 3:4])
    w_sb = sb(128, NCH, "w_sb")
    dve.scalar_tensor_tensor(out=w_sb[:], in0=alloc[:], scalar=scb[:, 2:3],
                             in1=t2[:], op0=OP.mult, op1=OP.add)

    # -------- w-derived operands for the L pass --------
    oww = bpool.tile([128, 2 * NCH], BF16, tag="oww", name="oww")
    oww3 = oww[:].rearrange("q (t i) -> q t i", i=NCH)
    dve.memset(oww3[:, 0, :], 1.0)
    dve.tensor_copy(oww3[:, 1, :], w_sb[:])

    wrow = bfat.tile([1, N], F32, tag="wrow", name="wrow", bufs=1)
    wrow_bf = bpool.tile([1, N], BF16, tag="wrow_bf", name="wrow_bf")
    w_bc = bfat.tile([128, N], BF16, tag="w_bc", name="w_bc")
    for g in range(4):
        pr = ps_small(1, 512)
        for j in range(4):
            c = 4 * g + j
            mm(pr[0:1, 128 * j:128 * (j + 1)], w_sb[:, c:c + 1], i128[:])
        dve.tensor_copy(wrow[0:1, 512 * g:512 * (g + 1)], pr[:])
        dve.tensor_copy(wrow_bf[0:1, 512 * g:512 * (g + 1)], pr[:])
        pb = ps_small(128, 512)
        mm(pb[:], ones_row_bf[:], wrow_bf[0:1, 512 * g:512 * (g + 1)])
        act.copy(w_bc[:, 512 * g:512 * (g + 1)], pb[:])

    wsum = sb(1, 1, "wsum")
    pws = ps_small(1, NCH)
    mm(pws[:], ones_col[:], w_sb[:])
    ws16 = sb(1, NCH, "ws16")
    dve.tensor_copy(ws16[:], pws[:])
    dve.tensor_reduce(wsum[:], ws16[:], axis=mybir.AxisListType.X, op=OP.add)

    psum_s = sb(1, 1, "psum_s")
    pT = sb(128, NCH, "pT")
    nc.sync.dma_start(
        pT[:].rearrange("q (c o) -> q c o", o=1),
        p_ap[b, 0:1, :].rearrange("o (c q) -> q c o", q=128))
    pps = ps_small(1, NCH)
    mm(pps[:], ones_col[:], pT[:])
    ps16 = sb(1, NCH, "ps16")
    dve.tensor_copy(ps16[:], pps[:])
    dve.tensor_reduce(psum_s[:], ps16[:], axis=mybir.AxisListType.X, op=OP.add)

    pw2 = sb(1, 2, "pw2")
    dve.tensor_copy(pw2[0:1, 0:1], psum_s[:])
    dve.tensor_copy(pw2[0:1, 1:2], wsum[:])
    pbx = ps_small(128, 2)
    mm(pbx[:], ones_row[:], pw2[:])
    pwb = sb(128, 2, "pwb")
    dve.tensor_copy(pwb[:], pbx[:])

    # -------- memory update (independent of L; overlaps the stream) --------
    # M_new = M * F + G with rank-1 F = 1 - w (x) e, G = w (x) v built on PE
    ev = bpool.tile([1, 2 * WD], F32, tag="ev", name="ev")
    dve.tensor_copy(ev[0:1, 0:WD], er_sg[:])
    dve.tensor_copy(ev[0:1, WD:2 * WD], v_sb[0:1, O_WV:O_WV + WD])
    FG = bfat.tile([128, NCH * 2 * WD], F32, tag="FG", name="FG",
                    bufs=1)
    FG3 = FG[:].rearrange("q (i w) -> q i w", w=2 * WD)
    for i in range(NCH):
        pt = ps_small(128, 2 * WD)
        mm(pt[:], wrow[0:1, 128 * i:128 * (i + 1)], ev[:])
        dve.scalar_tensor_tensor(out=FG3[:, i, 0:WD], in0=pt[:, 0:WD],
                                 scalar=-1.0, in1=ones64[:, 0:WD],
                                 op0=OP.mult, op1=OP.add)
        dve.tensor_copy(FG3[:, i, WD:2 * WD], pt[:, WD:2 * WD])

    Mn_sb = fat(128, NCH * WD, "Mn_sb")
    Mn3 = Mn_sb[:].rearrange("q (i w) -> q i w", w=WD)
    for i in range(NCH):
        g1 = scr64.tile([128, WD], F32, tag="g64", name="g64")
        gp.tensor_tensor(g1[:], M3[:, i, :], FG3[:, i, 0:WD], op=OP.mult)
        gp.tensor_tensor(Mn3[:, i, :], g1[:], FG3[:, i, WD:2 * WD], op=OP.add)

    mq2 = sb(128, NCH, "mq2")
    sq2 = scr_act.tile([128, NCH * WD], F32, tag="sact", name="sact")
    dve.tensor_tensor(sq2[:], Mn_sb[:], Mn_sb[:], op=OP.mult)
    dve.tensor_reduce(mq2[:], sq2[:].rearrange(
        "q (i w) -> q i w", w=WD), axis=mybir.AxisListType.X, op=OP.add)
    mn2 = sb(128, NCH, "mn2")
    act.activation(mn2[:], mq2[:], AF.Sqrt)
    dve.tensor_scalar_add(mn2[:], mn2[:], EPS)
    rn2 = sb(128, NCH, "rn2")
    dve.reciprocal(rn2[:], mn2[:])

    MnT = bfat.tile([64, NCH * 128], F32, tag="MnT", name="MnT",
                     bufs=1)
    MnT3 = MnT[:].rearrange("q (i c) -> q i c", c=128)
    for g in range(4):
        pt = ps_small(64, 512)
        for j in range(4):
            pe.transpose(pt[:, 128 * j:128 * (j + 1)], Mn3[:, 4 * g + j, :],
                         i128[:])
        act.copy(MnT[0:64, 512 * g:512 * (g + 1)], pt[:])

    # -------- read keys --------
    rk2 = sb(1, R, "rk2")
    for r in range(R):
        s64 = scr64.tile([128, WD], F32, tag="s64", name="s64")
        act.activation(s64[0:1, :], v_sb[0:1, O_RK + WD * r:O_RK + WD * (r + 1)],
                       AF.Square, accum_out=rk2[0:1, r:r + 1])
    rkn_n = sb(1, R, "rkn_n")
    act.activation(rkn_n[:], rk2[:], AF.Sqrt)
    srn = sb(1, R, "srn")
    dve.tensor_tensor(srn[:], rs_s[:], rkn_n[:], op=OP.mult)
    act.activation(srn[:], srn[:], AF.Copy, bias=EPS)
    rrec = sb(1, R, "rrec")
    dve.reciprocal(rrec[:], srn[:])
    rf = sb(1, R, "rf")
    dve.tensor_tensor(rf[:], rs_s[:], rrec[:], op=OP.mult)
    rkn = sb(1, R * WD, "rkn")
    dve.tensor_tensor(rkn[:].rearrange("o (r w) -> o r w", w=WD),
                      v_sb[0:1, O_RK:O_RK + R * WD]
                      .rearrange("o (r w) -> o r w", w=WD),
                      rf[:].rearrange("o (r w) -> o r w", w=1)
                      .broadcast_to([1, R, WD]),
                      op=OP.mult)
    rknT = sb(64, R, "rknT")
    ptk = ps_small(64, R)
    for r in range(R):
        mm(ptk[:, r:r + 1], rkn[0:1, WD * r:WD * (r + 1)], one_one[:])
    dve.tensor_copy(rknT[:], ptk[:])

    # -------- read content scores + per-head softmax pieces --------
    rsc = sb(128, R * NCH, "rsc")
    rsc3 = rsc[:].rearrange("q (r i) -> q r i", i=NCH)
    for i in range(NCH):
        pt = ps_small(128, R)
        mm(pt[:], MnT3[:, i, :], rknT[:])
        dve.tensor_scalar_mul(rsc3[:, :, i], pt[:], rn2[:, i:i + 1])
    rex = sb(128, R * NCH, "rex")
    rex3 = rex[:].rearrange("q (r i) -> q r i", i=NCH)
    res_s = sb(128, R, "res_s")
    for r in range(R):
        act.activation(rex3[:, r, :], rsc3[:, r, :], AF.Exp,
                       accum_out=res_s[:, r:r + 1])
    ptot = ps_small(R, 1)
    mm(ptot[:], res_s[:], ones_col[:])
    rec4 = sb(R, 1, "rec4")
    dve.reciprocal(rec4[:], ptot[:])
    prr = ps_small(1, R)
    mm(prr[:], rec4[:], i128[0:R, 0:R])
    rec_row = sb(1, R, "rec_row")
    dve.tensor_copy(rec_row[:], prr[:])

    return dict(oww3=oww3, w_bc=w_bc, pT=pT, pwb=pwb, w_sb=w_sb,
                modes=modes, rec_row=rec_row, rex3=rex3, Mn3=Mn3)


def _build_post(nc, pools, aps, b, st):
    """L streaming pass + temporal weights + read vectors."""
    (bpool, bfat, lpool, scr_ttr, scr_act, scr64, pss, pbig, lbf, consts) = pools
    ones_row, ones_col, one_one, i128, iota, ones64, ones_row_bf = consts
    l_ap, out_ap = aps['L'], aps['out']
    act = nc.scalar
    dve = nc.vector
    gp = nc.gpsimd
    pe = nc.tensor

    def mm(out, lhsT, rhs, start=True, stop=True):
        pe.matmul(out, lhsT, rhs, start=start, stop=stop)

    def ps_small(p_, f):
        return pss.tile([p_, f], F32, tag="pss", name="pss")

    def sb(p_, f, tag):
        return bpool.tile([p_, f], F32, tag=tag, name=tag)

    def fat(p_, f, tag):
        return bfat.tile([p_, f], F32, tag=tag, name=tag)

    oww3, w_bc, pT, pwb, w_sb = (st['oww3'], st['w_bc'], st['pT'], st['pwb'],
                                 st['w_sb'])
    modes, rec_row, rex3, Mn3 = (st['modes'], st['rec_row'], st['rex3'],
                                 st['Mn3'])

    # -------- the L pass: stream 16 row blocks of 1 MB --------
    cscw_ps = pbig.tile([2, N], F32, tag="cscw", name="cscw")
    rs0 = sb(128, NCH, "rs0")
    lw = sb(128, NCH, "lw")
    for i in range(NCH):
        lblk = lpool.tile([128, N], F32, tag="lblk", name="lblk")
        nc.sync.dma_start(lblk[:], l_ap[b, 128 * i:128 * (i + 1), :])
        lb = lbf.tile([128, N], BF16, tag="lbf", name="lbf")
        act.activation(lb[:], lblk[:], AF.Copy, accum_out=rs0[:, i:i + 1])
        for c in range(4):
            mm(cscw_ps[:, 512 * c:512 * (c + 1)], oww3[:, :, i],
               lb[:, 512 * c:512 * (c + 1)],
               start=(i == 0), stop=(i == NCH - 1))
        sT = scr_ttr.tile([128, N], BF16, tag="sttr", name="sttr")
        dve.scalar_tensor_tensor(out=sT[:], in0=lb[:], scalar=1.0,
                                 in1=w_bc[:], op0=OP.mult, op1=OP.mult,
                                 accum_out=lw[:, i:i + 1])

    # -------- temporal weights from the four L sums --------
    cscw_sb = bfat.tile([2, N], F32, tag="cscw_sb", name="cscw_sb",
                         bufs=1)
    act.copy(cscw_sb[:], cscw_ps[:])
    csT = sb(128, 2 * NCH, "csT")
    csT3 = csT[:].rearrange("q (i t) -> q i t", t=2)
    ptc = ps_small(128, 2 * NCH)
    for c in range(NCH):
        mm(ptc[:, 2 * c:2 * c + 2], cscw_sb[0:2, 128 * c:128 * (c + 1)],
           i128[0:2, 0:2])
    dve.tensor_copy(csT[:], ptc[:])
    cs0T = csT3[:, :, 0]
    cwT = csT3[:, :, 1]

    # rowsum_Lnew = rs0 - w*rs0 - Lw + w*(P_sum - p)
    pwb0 = pwb[:, 0:1].rearrange("q (a o) -> q a o", a=1).broadcast_to(
        [128, 1, NCH])[:, 0, :]
    r_t1 = sb(128, NCH, "r_t1")
    gp.tensor_tensor(r_t1[:], pwb0, pT[:], op=OP.subtract)
    r_t2 = sb(128, NCH, "r_t2")
    gp.tensor_tensor(r_t2[:], w_sb[:], r_t1[:], op=OP.mult)
    r_u1 = sb(128, NCH, "r_u1")
    gp.tensor_tensor(r_u1[:], w_sb[:], rs0[:], op=OP.mult)
    r_s1 = sb(128, NCH, "r_s1")
    gp.tensor_tensor(r_s1[:], rs0[:], r_u1[:], op=OP.subtract)
    r_s2 = sb(128, NCH, "r_s2")
    gp.tensor_tensor(r_s2[:], r_s1[:], lw[:], op=OP.subtract)
    rrow_f = sb(128, NCH, "rrow_f")
    gp.tensor_tensor(rrow_f[:], r_s2[:], r_t2[:], op=OP.add)
    ebw = sb(128, NCH, "ebw")
    ebw_s = sb(128, 1, "ebw_s")
    act.activation(ebw[:], rrow_f[:], AF.Exp, scale=1.0 / N, accum_out=ebw_s[:])

    # colsum_Lnew = cs0 - w*cs0 - cw + p*(W_sum - w)
    pwb1 = pwb[:, 1:2].rearrange("q (a o) -> q a o", a=1).broadcast_to(
        [128, 1, NCH])[:, 0, :]
    c_t1 = sb(128, NCH, "c_t1")
    gp.tensor_tensor(c_t1[:], pwb1, w_sb[:], op=OP.subtract)
    c_t2 = sb(128, NCH, "c_t2")
    gp.tensor_tensor(c_t2[:], pT[:], c_t1[:], op=OP.mult)
    c_u1 = sb(128, NCH, "c_u1")
    gp.tensor_tensor(c_u1[:], w_sb[:], cs0T, op=OP.mult)
    c_s1 = sb(128, NCH, "c_s1")
    gp.tensor_tensor(c_s1[:], cs0T, c_u1[:], op=OP.subtract)
    c_s2 = sb(128, NCH, "c_s2")
    gp.tensor_tensor(c_s2[:], c_s1[:], cwT, op=OP.subtract)
    crow_f = sb(128, NCH, "crow_f")
    gp.tensor_tensor(crow_f[:], c_s2[:], c_t2[:], op=OP.add)
    efw = sb(128, NCH, "efw")
    efw_s = sb(128, 1, "efw_s")
    act.activation(efw[:], crow_f[:], AF.Exp, scale=1.0 / N, accum_out=efw_s[:])

    pt = ps_small(1, 1)
    mm(pt[:], ebw_s[:], ones_col[:])
    rec_b = sb(1, 1, "rec_b")
    dve.reciprocal(rec_b[:], pt[:])
    pt = ps_small(1, 1)
    mm(pt[:], efw_s[:], ones_col[:])
    rec_f = sb(1, 1, "rec_f")
    dve.reciprocal(rec_f[:], pt[:])

    # per-head combine coefficients: b0 = modes[r,0]/Zbwd, b1 = modes[r,1]/Zc_r,
    # b2 = modes[r,2]/Zfwd  (softmax normalizers folded into the mode weights)
    bvec = sb(1, 3 * R, "bvec")
    dve.tensor_tensor(bvec[0:1, 0:R],
                      modes[:].rearrange("o (r t) -> o r t", t=3)[:, :, 0],
                      rec_b[0:1, 0:1].broadcast_to([1, R]), op=OP.mult)
    dve.tensor_tensor(bvec[0:1, R:2 * R],
                      modes[:].rearrange("o (r t) -> o r t", t=3)[:, :, 1],
                      rec_row[:], op=OP.mult)
    dve.tensor_tensor(bvec[0:1, 2 * R:3 * R],
                      modes[:].rearrange("o (r t) -> o r t", t=3)[:, :, 2],
                      rec_f[0:1, 0:1].broadcast_to([1, R]), op=OP.mult)
    pbv = ps_small(128, 3 * R)
    mm(pbv[:], ones_row[:], bvec[:])
    Bco = sb(128, 3 * R, "Bco")
    dve.tensor_copy(Bco[:], pbv[:])

    # read weights and read vectors
    rw_sb = sb(128, R * NCH, "rw_sb")
    rw3 = rw_sb[:].rearrange("q (r i) -> q r i", i=NCH)
    def bcast_col(col):
        return col.rearrange("q (a o) -> q a o", a=1).broadcast_to(
            [128, 1, NCH])[:, 0, :]

    for r in range(R):
        z3 = sb(128, NCH, "z3")
        act.activation(z3[:], efw[:], AF.Copy, scale=Bco[:, 2 * R + r:2 * R + r + 1])
        z2 = sb(128, NCH, "z2")
        gp.tensor_tensor(z2[:], rex3[:, r, :], bcast_col(Bco[:, R + r:R + r + 1]),
                         op=OP.mult)
        gp.tensor_tensor(z2[:], z2[:], z3[:], op=OP.add)
        gp.tensor_tensor(rw3[:, r, :], ebw[:], bcast_col(Bco[:, r:r + 1]),
                         op=OP.mult)
        gp.tensor_tensor(rw3[:, r, :], rw3[:, r, :], z2[:], op=OP.add)

    prv = pbig.tile([R, WD], F32, tag="prv", name="prv")
    rw_by_i = rw_sb[:].rearrange("q (r i) -> q i r", i=NCH)
    for i in range(NCH):
        mm(prv[:], rw_by_i[:, i, :], Mn3[:, i, :],
           start=(i == 0), stop=(i == NCH - 1))
    out_sb = sb(R, WD, "out_sb")
    dve.tensor_copy(out_sb[:], prv[:])
    nc.sync.dma_start(out_ap[b], out_sb[:])


def build_nc():
    nc = bacc.Bacc("TRN2", target_bir_lowering=False, debug=False)

    dr = {}
    dr['x'] = nc.dram_tensor("x", [BC, IN_D], F32, kind="ExternalInput").ap()
    dr['memory'] = nc.dram_tensor("memory", [BC, N, WD], F32,
                                  kind="ExternalInput").ap()
    dr['L'] = nc.dram_tensor("L", [BC, N, N], F32, kind="ExternalInput").ap()
    dr['p'] = nc.dram_tensor("p", [BC, 1, N], F32, kind="ExternalInput").ap()
    w1_ap = nc.dram_tensor("W1", [IN_D, H_D], F32, kind="ExternalInput").ap()
    b1_ap = nc.dram_tensor("b1", [1, H_D], F32, kind="ExternalInput").ap()
    w2_ap = nc.dram_tensor("W2", [H_D, IFACE], F32, kind="ExternalInput").ap()
    b2_ap = nc.dram_tensor("b2", [1, IFACE], F32, kind="ExternalInput").ap()
    iota_ap = nc.dram_tensor("iota_p1", [128, NCH], F32,
                             kind="ExternalInput").ap()
    i128_ap = nc.dram_tensor("i128", [128, 128], F32, kind="ExternalInput").ap()
    dr['out'] = nc.dram_tensor("out", [BC, R, WD], F32,
                               kind="ExternalOutput").ap()

    with tile.TileContext(nc) as tc, ExitStack() as ctx:
        persist = ctx.enter_context(tc.tile_pool(name="persist", bufs=1))
        bpool = ctx.enter_context(tc.tile_pool(name="bpool", bufs=2))
        bfat = ctx.enter_context(tc.tile_pool(name="bfat", bufs=2))
        lpool = ctx.enter_context(tc.tile_pool(name="lpool", bufs=3))
        scr_ttr = ctx.enter_context(tc.tile_pool(name="scr_ttr", bufs=1))
        scr_act = ctx.enter_context(tc.tile_pool(name="scr_act", bufs=1))
        lbf = ctx.enter_context(tc.tile_pool(name="lbf", bufs=10))
        scr64 = ctx.enter_context(tc.tile_pool(name="scr64", bufs=3))
        pss = ctx.enter_context(tc.tile_pool(name="pss", bufs=3, space="PSUM"))
        pbig = ctx.enter_context(tc.tile_pool(name="pbig", bufs=1,
                                              space="PSUM"))

        ones_row = persist.tile([1, 128], F32, tag="ones_row")
        nc.vector.memset(ones_row[:], 1.0)
        ones_col = persist.tile([128, 1], F32, tag="ones_col")
        nc.vector.memset(ones_col[:], 1.0)
        one_one = persist.tile([1, 1], F32, tag="one_one")
        nc.vector.memset(one_one[:], 1.0)
        i128 = persist.tile([128, 128], F32, tag="i128")
        nc.sync.dma_start(i128[:], i128_ap)
        iota = persist.tile([128, NCH], F32, tag="iota")
        nc.sync.dma_start(iota[:], iota_ap)
        ones64 = persist.tile([128, 2 * WD], F32, tag="ones64")
        nc.vector.memset(ones64[:], 1.0)
        ones_row_bf = persist.tile([1, 128], BF16, tag="ones_row_bf")
        nc.vector.memset(ones_row_bf[:], 1.0)

        w1_sb = persist.tile([128, 2, H_D], F32, tag="w1_sb")
        for c in range(2):
            nc.sync.dma_start(w1_sb[:, c, :], w1_ap[128 * c:128 * (c + 1), :])
        w2_sb = persist.tile([128, 4, IFACE], F32, tag="w2_sb")
        for c in range(4):
            nc.sync.dma_start(w2_sb[:, c, :], w2_ap[128 * c:128 * (c + 1), :])
        b1_sb = persist.tile([1, H_D], F32, tag="b1_sb")
        nc.sync.dma_start(b1_sb[:], b1_ap)
        b2_sb = persist.tile([1, IFACE], F32, tag="b2_sb")
        nc.sync.dma_start(b2_sb[:], b2_ap)

        aps = dict(dr)
        aps.update(w1_sb=w1_sb, w2_sb=w2_sb, b1_sb=b1_sb, b2_sb=b2_sb)
        pools = (bpool, bfat, lpool, scr_ttr, scr_act, scr64, pss, pbig, lbf,
                 (ones_row, ones_col, one_one, i128, iota, ones64,
                  ones_row_bf))
        sts = [_build_pre(nc, pools, aps, b) for b in range(BC)]
        for b in range(BC):
            _build_post(nc, pools, aps, b, sts[b])

    nc.compile()
    return nc


_NC_CACHE = []


def kernel(x, memory, L, p, W1, b1, W2, b2):
    x = np.ascontiguousarray(x, np.float32)
    memory = np.ascontiguousarray(memory, np.float32)
    L = np.ascontiguousarray(L, np.float32)
    p = np.ascontiguousarray(p, np.float32)
    W1 = np.ascontiguousarray(W1, np.float32)
    b1 = np.ascontiguousarray(b1, np.float32).reshape(1, H_D)
    W2 = np.ascontiguousarray(W2, np.float32)
    b2 = np.ascontiguousarray(b2, np.float32).reshape(1, IFACE)

    iota = (np.arange(N, dtype=np.float32).reshape(NCH, 128).T + 1.0).copy()
    i128 = np.eye(128, dtype=np.float32)

    if not _NC_CACHE:
        _NC_CACHE.append(build_nc())
    nc = _NC_CACHE[0]

    in_maps = []
    for c in range(NCORES):
        s = slice(BC * c, BC * (c + 1))
        in_maps.append({
            'x': x[s], 'memory': memory[s], 'L': L[s], 'p': p[s],
            'W1': W1, 'b1': b1, 'W2': W2, 'b2': b2,
            'iota_p1': iota, 'i128': i128,
        })

    res = run_bass_kernel_spmd(nc, in_maps, list(range(NCORES)))
    outs = [res.results[c]['out'].reshape(BC, 1, R * WD)
            for c in range(NCORES)]
    return np.concatenate(outs, axis=0)



# revision 72
# speedup vs baseline: 1.2190x; 1.2190x over previous
"""DNC forward (single step) on 8 NeuronCores — Bass/Tile kernel.

Data parallel: 16 batches -> 2 per core. Algebraic facts exploited (valid
for the prev_state==None path of the reference):

* prev_rw is uniform (1/N)  => fwd/bwd temporal read weights only need the
  row-sums and column-sums of L_new, never L_new itself.  With
  rowsum0 = L@1, Lw = L@w, colsum0 = 1@L, cw = w@L (w = write weights):
      rowsum_Lnew = (1-w)*rowsum0 - Lw + w*(sum(p) - p)
      colsum_Lnew = (1-w)*colsum0 - cw + p*(sum(w) - w)
  so L is streamed exactly once from HBM (the memory-bound roofline).
* var_phi / usage are constant across slots => argsort is the identity and
  allocation[n] = (1-u) * u^(n+1) with u = 1e-4 * prod_r(1 - free_gate_r/N).
* cosine attention normalizes the keys, so the write/read strengths cancel
  (up to the 1e-8 epsilon) — the softplus chains are dead code.
* v[:, 471:727] (output_vector) is unused: only 471 of W2's columns load.

Per 1 MB row-block of L (128 rows x 2048 cols) the work is spread over four
engines so each stays under the 2.9 us DMA time of the block:
  ACT:  f32->bf16 copy with accum  -> rowsum0        (1.9 us)
  DVE:  stt mult-accum cols 0:1024 -> Lw (low half)  (1.2 us)
  POOL: stt mult-accum cols 1024:  -> Lw (high half) (1.0 us)
  PE:   [1|w]^T @ block (psum acc) -> colsum0 / cw   (0.9 us)
Activation-table thrash is avoided by doing all Sigmoid/Tanh work for both
batches first (set 2), then switching once to the Exp/Ln set (set 6);
sqrt is computed as exp(-0.5*ln(x)).  All slot-indexed vectors use a
(128 partitions x 16 chunks) layout, slot = 128*chunk + partition.
"""
import numpy as np
from contextlib import ExitStack

import concourse.bass as bass
import concourse.bacc as bacc
import concourse.tile as tile
from concourse import mybir
from concourse.bass_utils import run_bass_kernel_spmd

F32 = mybir.dt.float32
BF16 = mybir.dt.bfloat16
U32 = mybir.dt.uint32
AF = mybir.ActivationFunctionType
OP = mybir.AluOpType

NCORES = 8
BC = 2                  # batches per core
N = 2048                # memory slots
NCH = N // 128          # 16 slot chunks
WD = 64                 # word size
R = 4                   # read heads
IN_D, H_D = 256, 512
IFACE = 727             # full interface width (727); only first 471 used
VUSE = 471              # used interface columns

# interface vector slice offsets (within the used 471)
O_RK, O_WK = 0, 260
O_ER, O_WV, O_FG, O_RM = 325, 389, 453, 459
EPS = 1e-8

POOL_SPLIT = True       # Lw high half on gpsimd (else full-width on DVE)


def build_nc():
    nc = bacc.Bacc("TRN2", target_bir_lowering=False, debug=False)

    x_ap = nc.dram_tensor("x", [BC, IN_D], F32, kind="ExternalInput").ap()
    mem_ap = nc.dram_tensor("memory", [BC, N, WD], F32,
                            kind="ExternalInput").ap()
    l_ap = nc.dram_tensor("L", [BC, N, N], F32, kind="ExternalInput").ap()
    p_ap = nc.dram_tensor("p", [BC, 1, N], F32, kind="ExternalInput").ap()
    w1_ap = nc.dram_tensor("W1", [IN_D, H_D], F32, kind="ExternalInput").ap()
    b1_ap = nc.dram_tensor("b1", [1, H_D], F32, kind="ExternalInput").ap()
    w2_ap = nc.dram_tensor("W2", [H_D, IFACE], F32, kind="ExternalInput").ap()
    b2_ap = nc.dram_tensor("b2", [1, IFACE], F32, kind="ExternalInput").ap()
    i128_ap = nc.dram_tensor("i128", [128, 128], F32,
                             kind="ExternalInput").ap()
    out_ap = nc.dram_tensor("out", [BC, R, WD], F32,
                            kind="ExternalOutput").ap()

    with tile.TileContext(nc) as tc, ExitStack() as ctx:
        persist = ctx.enter_context(tc.tile_pool(name="persist", bufs=1))
        pb2 = ctx.enter_context(tc.tile_pool(name="pb2", bufs=2))
        scr = ctx.enter_context(tc.tile_pool(name="scr", bufs=2))
        lpool = ctx.enter_context(tc.tile_pool(name="lpool", bufs=4))
        lbf = ctx.enter_context(tc.tile_pool(name="lbf", bufs=9))
        std = ctx.enter_context(tc.tile_pool(name="std", bufs=2))
        stp = ctx.enter_context(tc.tile_pool(name="stp", bufs=3))
        pss = ctx.enter_context(tc.tile_pool(name="pss", bufs=2,
                                             space="PSUM"))
        pcs = ctx.enter_context(tc.tile_pool(name="pcs", bufs=1,
                                             space="PSUM"))
        pfg = ctx.enter_context(tc.tile_pool(name="pfg", bufs=1,
                                             space="PSUM"))

        act = nc.scalar
        dve = nc.vector
        gp = nc.gpsimd
        pe = nc.tensor

        def mm(out, lhsT, rhs, start=True, stop=True):
            pe.matmul(out, lhsT, rhs, start=start, stop=stop)

        def ps(p_, f):
            return pss.tile([p_, f], F32, tag="pss", name="pss")

        def sb(p_, f, tag, dt=F32):
            return pb2.tile([p_, f], dt, tag=tag, name=tag)

        def scratch(p_, f, tag, dt=F32):
            return scr.tile([p_, f], dt, tag=tag, name=tag)

        def rsqrt_dve(dst, x, p_, f, a, bb, iters):
            """dst = 1/sqrt(x) on DVE only: seed y0 = a/x + b (range-fitted),
            then Newton y <- y*(1.5 - 0.5*x*y^2).

            Keeps Ln/Sqrt off the ACT engine so a single activation table
            set (exp_and_others) serves the whole program.
            """
            dve.reciprocal(dst, x)
            dve.tensor_scalar(dst, dst, a, bb, op0=OP.mult, op1=OP.add)
            tmp = scratch(p_, f, f"nrt{p_}x{f}")
            for _ in range(iters):
                dve.tensor_tensor(tmp[:p_, :f], dst, dst, op=OP.mult)
                dve.tensor_tensor(tmp[:p_, :f], tmp[:p_, :f], x, op=OP.mult)
                dve.tensor_scalar(tmp[:p_, :f], tmp[:p_, :f], -0.5, 1.5,
                                  op0=OP.mult, op1=OP.add)
                dve.tensor_tensor(dst, dst, tmp[:p_, :f], op=OP.mult)

        def sigmoid_dve(dst, src, p_, f):
            """dst = 1/(1+exp(-src)) via Exp + DVE add/recip (no Sigmoid
            table)."""
            act.activation(dst, src, AF.Exp, scale=-1.0)
            dve.tensor_scalar_add(dst, dst, 1.0)
            dve.reciprocal(dst, dst)

        # ---------------- consts + weights ----------------
        ones_row = persist.tile([1, 128], F32, tag="ones_row")
        dve.memset(ones_row[:], 1.0)
        ones_col = persist.tile([128, 1], F32, tag="ones_col")
        dve.memset(ones_col[:], 1.0)
        one_one = persist.tile([1, 1], F32, tag="one_one")
        dve.memset(one_one[:], 1.0)
        ones_row_bf = persist.tile([1, 128], BF16, tag="ones_row_bf")
        dve.memset(ones_row_bf[:], 1.0)
        one_one_bf = persist.tile([1, 1], BF16, tag="one_one_bf")
        dve.memset(one_one_bf[:], 1.0)
        i128 = persist.tile([128, 128], F32, tag="i128")
        nc.sync.dma_start(i128[:], i128_ap)
        i128_bf = persist.tile([128, 128], BF16, tag="i128_bf")
        dve.tensor_copy(i128_bf[:], i128[:])

        xrows = []
        for b in range(BC):
            xr = persist.tile([1, IN_D], F32, tag=f"x_{b}")
            nc.sync.dma_start(xr[:], x_ap[b:b + 1, :])
            xrows.append(xr)
        w1_sb = persist.tile([128, 2, H_D], F32, tag="w1_sb")
        for c in range(2):
            nc.sync.dma_start(w1_sb[:, c, :], w1_ap[128 * c:128 * (c + 1), :])
        b1_sb = persist.tile([1, H_D], F32, tag="b1_sb")
        nc.sync.dma_start(b1_sb[:], b1_ap)
        b2_sb = persist.tile([1, VUSE], F32, tag="b2_sb")
        nc.sync.dma_start(b2_sb[:], b2_ap[0:1, 0:VUSE])
        w2_sb = persist.tile([128, 4, VUSE], F32, tag="w2_sb")
        for c in range(4):
            nc.sync.dma_start(w2_sb[:, c, :],
                              w2_ap[128 * c:128 * (c + 1), 0:VUSE])
        # bf16 copies of the controller weights: 4x faster PE matmuls on the
        # write-weight critical path (v errors ~1e-3, well inside tolerance)
        w1_bf = persist.tile([128, 2, H_D], BF16, tag="w1_bf")
        for c in range(2):
            dve.tensor_copy(w1_bf[:, c, :], w1_sb[:, c, :])
        w2_bf = persist.tile([128, 4, VUSE], BF16, tag="w2_bf")
        for c in range(4):
            dve.tensor_copy(w2_bf[:, c, :], w2_sb[:, c, :])

        # DMA order matters: everything on the write-weight critical path
        # (W2, M0, p0) goes before the first L blocks; M1/p1 follow them.
        S = [dict(), dict()]

        def load_Mp(b):
            M_sb = sb(128, NCH * WD, f"M")
            M3 = M_sb[:].rearrange("q (i w) -> q i w", w=WD)
            nc.sync.dma_start(M3, mem_ap[b].rearrange("(i q) w -> q i w",
                                                      q=128))
            pT = sb(128, NCH, "pT")
            nc.sync.dma_start(
                pT[:].rearrange("q (c o) -> q c o", o=1),
                p_ap[b, 0:1, :].rearrange("o (c q) -> q c o", q=128))
            S[b].update(M_sb=M_sb, M3=M3, pT=pT)

        load_Mp(0)
        pre_lblk = {}
        for i in range(2):
            lblk = lpool.tile([128, N], F32, tag="lblk", name="lblk")
            nc.sync.dma_start(lblk[:], l_ap[0, 128 * i:128 * (i + 1), :])
            pre_lblk[i] = lblk
        load_Mp(1)

        # ---------------- phase A: controller + sigmoid/tanh/square --------
        def ctrl_A(b):
            st = S[b]
            ptx = ps(128, 2)
            for c in range(2):
                mm(ptx[:, c:c + 1], xrows[b][0:1, 128 * c:128 * (c + 1)],
                   one_one[:])
            xT = sb(128, 2, "xT", BF16)
            dve.tensor_copy(xT[:], ptx[:])

            h_ps = ps(1, H_D)
            for c in range(2):
                mm(h_ps[:], xT[:, c:c + 1], w1_bf[:, c, :],
                   start=(c == 0), stop=(c == 1))
            h_lin = sb(1, H_D, "h_lin")
            dve.tensor_tensor(h_lin[:], h_ps[:], b1_sb[:], op=OP.add)
            # tanh(x) = 1 - 2/(exp(2x)+1)  (keeps Tanh off the act tables)
            h_sb = sb(1, H_D, "h_sb")
            act.activation(h_sb[:], h_lin[:], AF.Exp, scale=2.0)
            dve.tensor_scalar_add(h_sb[:], h_sb[:], 1.0)
            dve.reciprocal(h_sb[:], h_sb[:])
            dve.tensor_scalar(h_sb[:], h_sb[:], -2.0, 1.0, op0=OP.mult,
                              op1=OP.add)

            pth = ps(128, 4)
            for c in range(4):
                mm(pth[:, c:c + 1], h_sb[0:1, 128 * c:128 * (c + 1)],
                   one_one[:])
            hT = sb(128, 4, "hT", BF16)
            dve.tensor_copy(hT[:], pth[:])

            v_ps = ps(1, VUSE)
            for c in range(4):
                mm(v_ps[:], hT[:, c:c + 1], w2_bf[:, c, :],
                   start=(c == 0), stop=(c == 3))
            v_sb = sb(1, VUSE, "v_sb")
            dve.tensor_tensor(v_sb[:], v_ps[:], b2_sb[:], op=OP.add)

            er_sg = sb(1, WD, "er_sg")
            sigmoid_dve(er_sg[:], v_sb[0:1, O_ER:O_ER + WD], 1, WD)
            fawg = sb(1, 6, "fawg")      # sigmoid of [fg(4), ag, wg]
            sigmoid_dve(fawg[:], v_sb[0:1, O_FG:O_FG + 6], 1, 6)

            s64 = scratch(1, WD, "s64")
            wk2 = sb(1, 1, "wk2")
            act.activation(s64[:], v_sb[0:1, O_WK:O_WK + WD], AF.Square,
                           accum_out=wk2[:])
            rk2 = sb(1, R, "rk2")
            for r in range(R):
                s64r = scratch(1, WD, "s64")
                act.activation(s64r[:], v_sb[0:1, WD * r:WD * (r + 1)],
                               AF.Square, accum_out=rk2[0:1, r:r + 1])

            fgN = sb(1, R, "fgN")
            act.activation(fgN[:], fawg[0:1, 0:4], AF.Copy,
                           scale=-1.0 / N, bias=1.0)
            fg2 = sb(1, 2, "fg2")
            dve.tensor_tensor(fg2[:], fgN[0:1, 0:2], fgN[0:1, 2:4],
                              op=OP.mult)
            prod = sb(1, 1, "prod")
            dve.tensor_tensor(prod[:], fg2[0:1, 0:1], fg2[0:1, 1:2],
                              op=OP.mult)
            omu = sb(1, 1, "omu")        # 1 - u,  u = 1e-4*prod
            act.activation(omu[:], prod[:], AF.Copy, scale=-1e-4, bias=1.0)
            c1 = sb(1, 1, "c1")          # wg*ag
            dve.tensor_tensor(c1[:], fawg[0:1, 5:6], fawg[0:1, 4:5],
                              op=OP.mult)
            c2 = sb(1, 1, "c2")          # wg*(1-ag) = wg - c1
            dve.tensor_tensor(c2[:], fawg[0:1, 5:6], c1[:], op=OP.subtract)
            st.update(v_sb=v_sb, er_sg=er_sg, wk2=wk2, rk2=rk2, prod=prod,
                      omu=omu, c1=c1, c2=c2)

        # ---------------- phase B: exp/ln addressing ----------------
        def addr_B(b):
            st = S[b]
            M_sb, M3, pT = st['M_sb'], st['M3'], st['pT']
            v_sb = st['v_sb']

            # M row norms: rn_w = 1/sqrt(msq) = exp(-0.5*ln(msq))
            sq1 = scratch(128, NCH * WD, "sqs")
            gp.tensor_tensor(sq1[:], M_sb[:], M_sb[:], op=OP.mult)
            msq = sb(128, NCH, "msq")
            dve.tensor_reduce(msq[:], sq1[:].rearrange("q (i w) -> q i w",
                                                       w=WD),
                              axis=mybir.AxisListType.X, op=OP.add)
            rn_w = sb(128, NCH, "rn_w")
            rsqrt_dve(rn_w[:], msq[:], 128, NCH, 0.3475, 0.6097, 4)
            wf = sb(1, 1, "wf")          # 1/||write_key||
            rsqrt_dve(wf[:], st['wk2'][:], 1, 1, 1.93, 0.0611, 5)
            kn = sb(1, WD, "kn")
            act.activation(kn[:], v_sb[0:1, O_WK:O_WK + WD], AF.Copy,
                           scale=wf[:])
            pkb = ps(128, WD)
            mm(pkb[:], ones_row[:], kn[:])
            kn_bc = sb(128, WD, "kn_bc")
            dve.tensor_copy(kn_bc[:], pkb[:])

            # write content scores (gpsimd dots), softmax over 2048 slots
            wsc_r = sb(128, NCH, "wsc_r")
            for i in range(NCH):
                g64 = scratch(128, WD, "g64")
                dve.scalar_tensor_tensor(out=g64[:], in0=M3[:, i, :],
                                         scalar=1.0, in1=kn_bc[:],
                                         op0=OP.mult, op1=OP.mult,
                                         accum_out=wsc_r[:, i:i + 1])
            wsc = sb(128, NCH, "wsc")
            dve.tensor_tensor(wsc[:], wsc_r[:], rn_w[:], op=OP.mult)
            wse = sb(128, NCH, "wse")
            wse_s = sb(128, 1, "wse_s")
            act.activation(wse[:], wsc[:], AF.Exp, accum_out=wse_s[:])
            ptt = ps(1, 1)
            mm(ptt[:], wse_s[:], ones_col[:])
            totr = sb(1, 1, "totr")
            dve.reciprocal(totr[:], ptt[:])

            # write weights: w = wg*(1-ag)*content_softmax everywhere; slot 0
            # additionally gets wg*ag*u*(1-u)  (allocation = (1-u)*u^(n+1)
            # with u <= 1e-4, so every n >= 1 term is < 1e-8 and drops out)
            c2r = sb(1, 1, "c2r")
            dve.tensor_tensor(c2r[:], st['c2'][:], totr[:], op=OP.mult)
            pc1 = ps(128, 1)
            mm(pc1[:], ones_row[:], c2r[:])
            c2c = sb(128, 1, "c2c")
            dve.tensor_copy(c2c[:], pc1[:])
            w_sb = sb(128, NCH, "w_sb")
            dve.tensor_scalar_mul(w_sb[:], wse[:], c2c[:])
            u_t = sb(1, 1, "u_t")
            dve.tensor_scalar_mul(u_t[:], st['prod'][:], 1e-4)
            uom = sb(1, 1, "uom")
            dve.tensor_tensor(uom[:], u_t[:], st['omu'][:], op=OP.mult)
            v1 = sb(1, 1, "v1")
            dve.tensor_tensor(v1[:], uom[:], st['c1'][:], op=OP.mult)
            dve.tensor_tensor(w_sb[0:1, 0:1], w_sb[0:1, 0:1], v1[:],
                              op=OP.add)
            w16 = sb(128, NCH, "w16", BF16)
            dve.tensor_copy(w16[:], w_sb[:])

            # P = sum(p), W = sum(w) broadcast to columns
            pps = ps(1, NCH)
            mm(pps[:], ones_col[:], pT[:])
            P_s = sb(1, 1, "P_s")
            dve.tensor_reduce(P_s[:], pps[:], axis=mybir.AxisListType.X,
                              op=OP.add)
            pws = ps(1, NCH)
            mm(pws[:], ones_col[:], w_sb[:])
            W_s = sb(1, 1, "W_s")
            dve.tensor_reduce(W_s[:], pws[:], axis=mybir.AxisListType.X,
                              op=OP.add)
            sc2 = sb(1, 2, "sc2")
            dve.tensor_copy(sc2[0:1, 0:1], P_s[:])
            dve.tensor_copy(sc2[0:1, 1:2], W_s[:])
            pb2m = ps(128, 2)
            mm(pb2m[:], ones_row[:], sc2[:])
            scb2 = sb(128, 2, "scb2")
            dve.tensor_copy(scb2[:], pb2m[:])

            # oww[:, i, :] = [1 | w chunk i]  (cscw matmul lhsT)
            oww = sb(128, 2 * NCH, "oww", BF16)
            oww3 = oww[:].rearrange("q (i t) -> q i t", t=2)
            dve.memset(oww3[:, :, 0], 1.0)
            dve.tensor_copy(oww3[:, :, 1], w16[:].rearrange(
                "q (i o) -> q i o", o=1)[:, :, 0])

            # w as a bf16 row [1, N] (slot-major), then broadcast to 128 rows
            wrow_bf = sb(1, N, "wrow_bf", BF16)
            for g in range(4):
                prow = ps(1, 512)
                for j in range(4):
                    c = 4 * g + j
                    mm(prow[0:1, 128 * j:128 * (j + 1)], w16[:, c:c + 1],
                       i128_bf[:])
                act.copy(wrow_bf[0:1, 512 * g:512 * (g + 1)], prow[:])
            w_bc = sb(128, N, "w_bc", BF16)
            for g in range(4):
                pwb = ps(128, 512)
                mm(pwb[:], ones_row_bf[:], wrow_bf[0:1, 512 * g:512 * (g + 1)])
                dve.tensor_copy(w_bc[:, 512 * g:512 * (g + 1)], pwb[:])

            # memory update:  Mn = M*(1 - w(x)e) + w(x)v,  via psum outer
            # products [w(x)(-e) | w(x)v] and fused (1+F)*M + G on DVE
            ev = sb(1, 2 * WD, "ev", BF16)
            act.activation(ev[0:1, 0:WD], st['er_sg'], AF.Copy, scale=-1.0)
            dve.tensor_copy(ev[0:1, WD:2 * WD], v_sb[0:1, O_WV:O_WV + WD])
            Mn_sb = sb(128, NCH * WD, "Mn")
            Mn3 = Mn_sb[:].rearrange("q (i w) -> q i w", w=WD)
            for half in range(2):
                pf = pfg.tile([128, 8 * 2 * WD], F32, tag="pfg", name="pfg")
                pf3 = pf[:].rearrange("q (i w) -> q i w", w=2 * WD)
                for j in range(8):
                    i = 8 * half + j
                    mm(pf3[:, j, :], wrow_bf[0:1, 128 * i:128 * (i + 1)],
                       ev[:])
                th = scratch(128, 8 * WD, "th")
                th3 = th[:].rearrange("q (i w) -> q i w", w=WD)
                dve.scalar_tensor_tensor(
                    out=th3[:, :, :], in0=pf3[:, :, 0:WD], scalar=1.0,
                    in1=M3[:, 8 * half:8 * (half + 1), :],
                    op0=OP.add, op1=OP.mult)
                dve.tensor_tensor(Mn3[:, 8 * half:8 * (half + 1), :],
                                  th3[:, :, :], pf3[:, :, WD:2 * WD],
                                  op=OP.add)

            # Mn row norms -> rn2, scaled copy Mn_s = Mn * rn2 (per slot)
            sq2 = scratch(128, NCH * WD, "sqs")
            gp.tensor_tensor(sq2[:], Mn_sb[:], Mn_sb[:], op=OP.mult)
            mq2 = sb(128, NCH, "mq2")
            dve.tensor_reduce(mq2[:], sq2[:].rearrange("q (i w) -> q i w",
                                                       w=WD),
                              axis=mybir.AxisListType.X, op=OP.add)
            rn2 = sb(128, NCH, "rn2")
            rsqrt_dve(rn2[:], mq2[:], 128, NCH, 0.3475, 0.6097, 4)
            Mn_s = scratch(128, NCH * WD, "sqs")
            Mn_s3 = Mn_s[:].rearrange("q (i w) -> q i w", w=WD)
            dve.tensor_tensor(
                Mn_s3[:, :, :], Mn3[:, :, :],
                rn2[:].rearrange("q (i o) -> q i o", o=1)
                .broadcast_to([128, NCH, WD]), op=OP.mult)

            # transpose Mn_s -> MnT_s (bf16) for read content scores
            MnT_s = sb(64, NCH * 128, "MnT_s", BF16)
            for g in range(4):
                pt = ps(64, 512)
                for j in range(4):
                    pe.transpose(pt[:, 128 * j:128 * (j + 1)],
                                 Mn_s3[:, 4 * g + j, :], i128[:])
                act.copy(MnT_s[0:64, 512 * g:512 * (g + 1)], pt[:])

            # normalized read keys -> rknT (bf16)
            rf = sb(1, R, "rf")
            rsqrt_dve(rf[:], st['rk2'][:], 1, R, 1.93, 0.0611, 5)
            rkn = sb(1, R * WD, "rkn", BF16)
            dve.tensor_tensor(rkn[:].rearrange("o (r w) -> o r w", w=WD),
                              v_sb[0:1, O_RK:O_RK + R * WD]
                              .rearrange("o (r w) -> o r w", w=WD),
                              rf[:].rearrange("o (r w) -> o r w", w=1)
                              .broadcast_to([1, R, WD]), op=OP.mult)
            prk = ps(64, R)
            for r in range(R):
                mm(prk[:, r:r + 1], rkn[0:1, WD * r:WD * (r + 1)],
                   one_one_bf[:])
            rknT = sb(64, R, "rknT", BF16)
            dve.tensor_copy(rknT[:], prk[:])

            # read content scores + per-head exp/softmax partials
            prsc = ps(128, R * NCH)
            for i in range(NCH):
                mm(prsc[:, R * i:R * (i + 1)],
                   MnT_s[0:64, 128 * i:128 * (i + 1)], rknT[:])
            rex = sb(128, R * NCH, "rex")
            rex3 = rex[:].rearrange("q (r i) -> q r i", i=NCH)
            res_s = sb(128, R, "res_s")
            prsc3 = prsc[:].rearrange("q (i r) -> q r i", r=R)
            for r in range(R):
                act.activation(rex3[:, r, :], prsc3[:, r, :], AF.Exp,
                               accum_out=res_s[:, r:r + 1])
            prt = ps(R, 1)
            mm(prt[:], res_s[:], ones_col[:])
            rec4 = sb(R, 1, "rec4")
            dve.reciprocal(rec4[:], prt[:])
            prr = ps(1, R)
            mm(prr[:], rec4[:], i128[0:R, 0:R])
            rec_row = sb(1, R, "rec_row")
            dve.tensor_copy(rec_row[:], prr[:])

            # read modes softmax (over 3) scaled by softmax normalizers later
            rm_e = sb(1, 3 * R, "rm_e")
            act.activation(rm_e[:], v_sb[0:1, O_RM:O_RM + 3 * R], AF.Exp)
            rm_sum = sb(1, R, "rm_sum")
            dve.tensor_reduce(rm_sum[:], rm_e[:].rearrange(
                "o (r t) -> o r t", t=3), axis=mybir.AxisListType.X,
                op=OP.add)
            rm_rec = sb(1, R, "rm_rec")
            dve.reciprocal(rm_rec[:], rm_sum[:])
            modes = sb(1, 3 * R, "modes")
            dve.tensor_tensor(modes[:].rearrange("o (r t) -> o r t", t=3),
                              rm_e[:].rearrange("o (r t) -> o r t", t=3),
                              rm_rec[:].rearrange("o (r t) -> o r t", t=1)
                              .broadcast_to([1, R, 3]), op=OP.mult)

            st.update(w_sb=w_sb, oww3=oww3, w_bc=w_bc, Mn3=Mn3, rex3=rex3,
                      rec_row=rec_row, modes=modes, scb2=scb2)

        # ---------------- L stream ----------------
        def stream_head(b, k, pre=None):
            """DMA + ACT convert for the first k blocks, emitted before the
            controller so the converts sit at the head of the ACT queue."""
            st = S[b]
            rs0 = sb(128, NCH, "rs0")
            lwd = sb(128, NCH, "lwd")
            lwp = sb(128, NCH, "lwp")
            heads = []
            for i in range(k):
                if pre is not None and i in pre:
                    lblk = pre[i]
                else:
                    lblk = lpool.tile([128, N], F32, tag="lblk",
                                      name="lblk")
                    nc.sync.dma_start(lblk[:],
                                      l_ap[b, 128 * i:128 * (i + 1), :])
                lb = lbf.tile([128, N], BF16, tag="lbf", name="lbf")
                act.activation(lb[:], lblk[:], AF.Copy,
                               accum_out=rs0[:, i:i + 1])
                heads.append(lb)
            st.update(rs0=rs0, lwd=lwd, lwp=lwp, heads=heads)

        def stream(b, weave=None):
            st = S[b]
            oww3 = st['oww3']
            w_bc = st['w_bc']
            rs0, lwd, lwp = st['rs0'], st['lwd'], st['lwp']
            heads = st['heads']
            cscw = pcs.tile([2, N], F32, tag="cscw", name="cscw")
            for i in range(NCH):
                if i < len(heads):
                    lb = heads[i]
                else:
                    lblk = lpool.tile([128, N], F32, tag="lblk",
                                      name="lblk")
                    nc.sync.dma_start(lblk[:],
                                      l_ap[b, 128 * i:128 * (i + 1), :])
                    lb = lbf.tile([128, N], BF16, tag="lbf", name="lbf")
                    act.activation(lb[:], lblk[:], AF.Copy,
                                   accum_out=rs0[:, i:i + 1])
                sTd = std.tile([128, 1024], BF16, tag="sTd", name="sTd")
                dve.scalar_tensor_tensor(out=sTd[:], in0=lb[:, 0:1024],
                                         scalar=1.0, in1=w_bc[:, 0:1024],
                                         op0=OP.mult, op1=OP.mult,
                                         accum_out=lwd[:, i:i + 1])
                # hi half: multiply on gpsimd (plain TensorTensor is the only
                # elementwise opcode the Pool engine supports), reduce on
                # alternating ACT / DVE so no engine exceeds the DMA pace
                sTp = stp.tile([128, 1024], BF16, tag="sTp", name="sTp")
                gp.tensor_tensor(sTp[:], lb[:, 1024:2048],
                                 w_bc[:, 1024:2048], op=OP.mult)
                if i % 2 == 0:
                    act.activation(sTp[:], sTp[:], AF.Copy,
                                   accum_out=lwp[:, i:i + 1])
                else:
                    dve.tensor_reduce(
                        lwp[:, i:i + 1],
                        sTp[:].rearrange("q (a w) -> q a w", a=1),
                        axis=mybir.AxisListType.X, op=OP.add)
                if i < NCH - 1:
                    # row-form colsum/cw accumulation for blocks 0..14
                    for c in range(4):
                        mm(cscw[:, 512 * c:512 * (c + 1)], oww3[:, i, :],
                           lb[:, 512 * c:512 * (c + 1)],
                           start=(i == 0), stop=(i == NCH - 2))
                else:
                    st.update(last_lb=lb)
                if weave is not None and i in weave:
                    weave[i]()
            st.update(rs0=rs0, lwd=lwd, cscw=cscw)

        # ---------------- finalize: temporal weights + read vectors --------
        def finalize_cs(b):
            """Extract colsum0/cw: transpose the 15-block row-form psum, then
            accumulate the last block's contribution directly in transposed
            form (16 tiny matmuls) so the [2, 2048] copy stays off the tail.
            """
            st = S[b]
            cscw = st['cscw']
            oww3, last_lb = st['oww3'], st['last_lb']
            pcst = ps(128, 2 * NCH)
            for g in range(4):
                seg = slice(512 * g, 512 * (g + 1))
                cseg = scratch(2, 512, f"cseg{g % 2}")
                dve.tensor_copy(cseg[:], cscw[:, seg])
                for j in range(4):
                    c = 4 * g + j
                    mm(pcst[:, 2 * c:2 * c + 2],
                       cseg[0:2, 128 * j:128 * (j + 1)], i128[0:2, 0:2],
                       start=True, stop=False)
            for c in range(NCH):
                mm(pcst[:, 2 * c:2 * c + 2],
                   last_lb[:, 128 * c:128 * (c + 1)], oww3[:, c, :],
                   start=False, stop=True)
            csT = sb(128, 2 * NCH, "csT")
            dve.tensor_copy(csT[:], pcst[:])
            st.update(csT=csT)

        def finalize(b):
            st = S[b]
            rs0, lwd, lwp = st['rs0'], st['lwd'], st['lwp']
            pT, w_sb, scb2 = st['pT'], st['w_sb'], st['scb2']
            rex3, rec_row, modes, Mn3 = (st['rex3'], st['rec_row'],
                                         st['modes'], st['Mn3'])
            csT3 = st['csT'][:].rearrange("q (c t) -> q c t", t=2)
            cs0T = csT3[:, :, 0]
            cwT = csT3[:, :, 1]

            # rowsum_Lnew = rs0 - w*(rs0 + p - P) - Lw   (per slot)
            # row side on DVE, col side on Pool: the two tail chains overlap
            lwT = sb(128, NCH, "lwT")
            dve.tensor_tensor(lwT[:], lwd[:], lwp[:], op=OP.add)
            D = sb(128, NCH, "Dt")
            dve.tensor_tensor(D[:], rs0[:], pT[:], op=OP.add)
            E = sb(128, NCH, "Et")
            dve.scalar_tensor_tensor(out=E[:], in0=D[:],
                                     scalar=scb2[:, 0:1], in1=w_sb[:],
                                     op0=OP.subtract, op1=OP.mult)
            Fm = sb(128, NCH, "Fm")
            dve.tensor_tensor(Fm[:], rs0[:], lwT[:], op=OP.subtract)
            rrow = sb(128, NCH, "rrow")
            dve.tensor_tensor(rrow[:], Fm[:], E[:], op=OP.subtract)
            # colsum_Lnew = cs0 - w*cs0 - cw - p*(w - W)
            H = sb(128, NCH, "Ht")
            dve.scalar_tensor_tensor(out=H[:], in0=w_sb[:],
                                     scalar=scb2[:, 1:2], in1=pT[:],
                                     op0=OP.subtract, op1=OP.mult)
            K = sb(128, NCH, "Kt")
            gp.tensor_tensor(K[:], w_sb[:], cs0T, op=OP.mult)
            J = sb(128, NCH, "Jt")
            gp.tensor_tensor(J[:], cs0T, cwT, op=OP.subtract)
            L1 = sb(128, NCH, "L1t")
            gp.tensor_tensor(L1[:], J[:], K[:], op=OP.subtract)
            crow = sb(128, NCH, "crow")
            gp.tensor_tensor(crow[:], L1[:], H[:], op=OP.subtract)

            ebw = sb(128, NCH, "ebw")
            ebw_s = sb(128, 1, "ebw_s")
            act.activation(ebw[:], rrow[:], AF.Exp, scale=1.0 / N,
                           accum_out=ebw_s[:])
            efw = sb(128, NCH, "efw")
            efw_s = sb(128, 1, "efw_s")
            act.activation(efw[:], crow[:], AF.Exp, scale=1.0 / N,
                           accum_out=efw_s[:])
            pzb = ps(1, 1)
            mm(pzb[:], ebw_s[:], ones_col[:])
            rec_b = sb(1, 1, "rec_b")
            dve.reciprocal(rec_b[:], pzb[:])
            pzf = ps(1, 1)
            mm(pzf[:], efw_s[:], ones_col[:])
            rec_f = sb(1, 1, "rec_f")
            dve.reciprocal(rec_f[:], pzf[:])

            # per-head combine coefficients (softmax normalizers folded in)
            m3 = modes[:].rearrange("o (r t) -> o r t", t=3)
            bvec = sb(1, 3 * R, "bvec")
            dve.tensor_tensor(bvec[0:1, 0:R], m3[:, :, 0],
                              rec_b[0:1, 0:1].broadcast_to([1, R]),
                              op=OP.mult)
            dve.tensor_tensor(bvec[0:1, R:2 * R], m3[:, :, 1], rec_row[:],
                              op=OP.mult)
            dve.tensor_tensor(bvec[0:1, 2 * R:3 * R], m3[:, :, 2],
                              rec_f[0:1, 0:1].broadcast_to([1, R]),
                              op=OP.mult)
            pbv = ps(128, 3 * R)
            mm(pbv[:], ones_row[:], bvec[:])
            Bco = sb(128, 3 * R, "Bco")
            dve.tensor_copy(Bco[:], pbv[:])

            rw_sb = sb(128, R * NCH, "rw_sb")
            rw3 = rw_sb[:].rearrange("q (r i) -> q r i", i=NCH)
            for r in range(R):
                z3 = sb(128, NCH, "z3")
                dve.tensor_scalar_mul(z3[:], efw[:],
                                      Bco[:, 2 * R + r:2 * R + r + 1])
                z2 = sb(128, NCH, "z2")
                dve.scalar_tensor_tensor(out=z2[:], in0=rex3[:, r, :],
                                         scalar=Bco[:, R + r:R + r + 1],
                                         in1=z3[:], op0=OP.mult, op1=OP.add)
                dve.scalar_tensor_tensor(out=rw3[:, r, :], in0=ebw[:],
                                         scalar=Bco[:, r:r + 1], in1=z2[:],
                                         op0=OP.mult, op1=OP.add)

            prv = ps(R, WD)
            rw_by_i = rw_sb[:].rearrange("q (r i) -> q i r", i=NCH)
            for i in range(NCH):
                mm(prv[:], rw_by_i[:, i, :], Mn3[:, i, :],
                   start=(i == 0), stop=(i == NCH - 1))
            out_sb = sb(R, WD, "out_sb")
            dve.tensor_copy(out_sb[:], prv[:])
            st.update(out_sb=out_sb)

        # ---------------- emission schedule ----------------
        ctrl_A(0)
        addr_B(0)
        ctrl_A(1)
        stream_head(0, 2, pre=pre_lblk)
        stream(0, weave={8: (lambda: addr_B(1))})
        finalize_cs(0)
        stream_head(1, 0)
        stream(1, weave={2: (lambda: finalize(0))})
        finalize_cs(1)
        finalize(1)
        for b in range(BC):
            nc.sync.dma_start(out_ap[b], S[b]['out_sb'][:])

    nc.compile()
    return nc


_NC_CACHE = []


def kernel(x, memory, L, p, W1, b1, W2, b2):
    x = np.ascontiguousarray(x, np.float32)
    memory = np.ascontiguousarray(memory, np.float32)
    L = np.ascontiguousarray(L, np.float32)
    p = np.ascontiguousarray(p, np.float32)
    W1 = np.ascontiguousarray(W1, np.float32)
    b1 = np.ascontiguousarray(b1, np.float32).reshape(1, H_D)
    W2 = np.ascontiguousarray(W2, np.float32)
    b2 = np.ascontiguousarray(b2, np.float32).reshape(1, IFACE)

    i128 = np.eye(128, dtype=np.float32)

    if not _NC_CACHE:
        _NC_CACHE.append(build_nc())
    nc = _NC_CACHE[0]

    in_maps = []
    for c in range(NCORES):
        s = slice(BC * c, BC * (c + 1))
        in_maps.append({
            'x': x[s], 'memory': memory[s], 'L': L[s], 'p': p[s],
            'W1': W1, 'b1': b1, 'W2': W2, 'b2': b2,
            'i128': i128,
        })

    res = run_bass_kernel_spmd(nc, in_maps, list(range(NCORES)))
    outs = [res.results[c]['out'].reshape(BC, 1, R * WD)
            for c in range(NCORES)]
    return np.concatenate(outs, axis=0)
